# revision 1
# baseline (speedup 1.0000x reference)
"""Trainium2 Bass kernel for nn_ConvLayer_56453050139435.

Reference computation (StyleGAN2-style downsampling conv layer):
  1. depthwise 4x4 binomial blur ([1,3,3,1] outer [1,3,3,1] / 64) with pad 2
  2. 3x3 stride-2 conv, 128 -> 256 channels, weight scaled by 1/sqrt(fan_in)
  3. bias + leaky-relu(0.2) * sqrt(2), clamp +-256 (never binds: |out| < ~4)

Sharding: data-parallel over batch, 2 images per core across 8 cores.

Per-core pipeline (fp16 data path, fp32 PSUM accumulation), fully
block-streamed: each block of 8 output rows owns a 20-row strip of the
input (4-row halo recomputed between blocks) so DMA / ACT / DVE / PE
pipeline at block granularity with small pool-rotated tiles:
  - horizontal blur on DVE (2 shifted adds + scalar_tensor_tensor), using an
    ACT-engine shifted copy so every DVE operand is 4-byte aligned (2x mode)
  - vertical blur on DVE (aligned row-pitch shifts)
  - conv as 9-tap matmul accumulation in PSUM, oc split in two 128-halves,
    rhs = stride-2 access pattern on the blurred strip
  - epilogue: one ACT Prelu op (scale=sqrt2, bias, alpha=0.2) PSUM->SBUF fp16
  - DMA out fp16, host-cast back to fp32
"""

import numpy as np

import concourse.bass as bass
import concourse.mybir as mybir
from concourse import bacc
from concourse.tile import TileContext
from concourse.bass_utils import run_bass_kernel_spmd

AF = mybir.ActivationFunctionType
OP = mybir.AluOpType
FP16 = mybir.dt.float16
FP32 = mybir.dt.float32

IC, OC, H, W = 128, 256, 256, 256
OH, OW = 128, 128
KS = 3
N_CORES = 8
B_PER_CORE = 2
SQRT2 = float(np.sqrt(2.0))
WSCALE = 1.0 / float(np.sqrt(KS * KS * IC))
LRELU_SLOPE = 0.2

XPITCH = 264     # padded x row pitch: x col w lives at buffer col w+2
HB_W = 260       # blur cols 0..256 valid, 257+ garbage (never consumed)
SROWS = 36       # hb rows per pair of blocks (32 vb rows + 4-row halo)
NPAIR = 8        # pairs of 8-output-row blocks per image


def _build_nc():
    nc = bacc.Bacc(None, target_bir_lowering=False)
    x_d = nc.dram_tensor("x", [B_PER_CORE, IC, H, W], FP16, kind="ExternalInput")
    w_d = nc.dram_tensor("w", [IC, 18 * 128], FP16, kind="ExternalInput")
    b_d = nc.dram_tensor("b", [128, 2], FP32, kind="ExternalInput")
    y_d = nc.dram_tensor("y", [B_PER_CORE, OC, OH, OW], FP16, kind="ExternalOutput")

    with TileContext(nc) as tc:
        with (
            tc.tile_pool(name="const", bufs=1) as cpool,
            tc.tile_pool(name="xin", bufs=2) as xpool,
            tc.tile_pool(name="shift", bufs=1) as spool,
            tc.tile_pool(name="scr", bufs=1) as scrpool,
            tc.tile_pool(name="hb", bufs=1) as hbpool,
            tc.tile_pool(name="vb", bufs=3) as vbpool,
            tc.tile_pool(name="out", bufs=4) as opool,
            tc.tile_pool(name="psum", bufs=8, space="PSUM") as pspool,
        ):
            wt = cpool.tile([128, 18 * 128], FP16)
            bt = cpool.tile([128, 2], FP32)
            al = cpool.tile([128, 1], FP32)
            nc.sync.dma_start(wt[:], w_d[:])
            nc.sync.dma_start(bt[:], b_d[:])
            nc.vector.memset(al[:], LRELU_SLOPE)

            # DVE-only scratch: single-buffered (engine order serializes)
            t1 = scrpool.tile([128, SROWS, HB_W], FP16)
            t2 = scrpool.tile([128, SROWS, HB_W], FP16)
            t3 = scrpool.tile([128, SROWS, HB_W], FP16)
            vu = scrpool.tile([128, 17, HB_W], FP16)
            vw = scrpool.tile([128, 17, HB_W], FP16)

            for img in range(B_PER_CORE):
                for P in range(NPAIR):
                    lo = 32 * P - 2       # x row of hb tile row 0
                    xr0 = max(lo, 0)
                    xr1 = min(lo + SROWS, H)
                    ta, tb = xr0 - lo, xr1 - lo  # valid hb tile row range

                    xt = xpool.tile([128, SROWS, XPITCH], FP16)
                    # x cols -2..-1 and 256..257 must be zero; cols beyond are
                    # garbage that only feeds hb cols >256 (never consumed)
                    nc.gpsimd.memset(xt[:, ta:tb, 0:2], 0.0)
                    nc.gpsimd.memset(xt[:, ta:tb, 258:260], 0.0)
                    nc.sync.dma_start(
                        xt[:, ta:tb, 2:258], x_d[img, :, xr0:xr1, :]
                    )
                    # shifted copy (ACT): ct[j] = xt[j+1], keeps DVE aligned
                    ct = spool.tile([128, SROWS, 262], FP16)
                    nc.scalar.copy(ct[:, ta:tb, :], xt[:, ta:tb, 1:263])

                    hb = hbpool.tile([128, SROWS, HB_W], FP16)
                    if ta > 0:
                        nc.gpsimd.memset(hb[:, 0:ta, :], 0.0)
                    if tb < SROWS:
                        nc.gpsimd.memset(hb[:, tb:SROWS, :], 0.0)
                    # hb[c] = x[c-2] + 3x[c-1] + 3x[c] + x[c+1]
                    #       = (xt[c] + ct[c+2]) + 3*(ct[c] + xt[c+2])
                    nc.vector.tensor_tensor(
                        out=t1[:, ta:tb, :], in0=xt[:, ta:tb, 0:HB_W],
                        in1=ct[:, ta:tb, 2:262], op=OP.add,
                    )
                    nc.vector.tensor_tensor(
                        out=t2[:, ta:tb, :], in0=ct[:, ta:tb, 0:HB_W],
                        in1=xt[:, ta:tb, 2 : 2 + HB_W], op=OP.add,
                    )
                    nc.vector.tensor_scalar_mul(t3[:, ta:tb, :], t2[:, ta:tb, :], 3.0)
                    nc.vector.tensor_tensor(
                        out=hb[:, ta:tb, :], in0=t1[:, ta:tb, :],
                        in1=t3[:, ta:tb, :], op=OP.add,
                    )
                    # t3 <- 3*hb (full 36 rows: edge rows of hb are zeroed)
                    nc.vector.tensor_scalar_mul(t3[:], hb[:], 3.0)

                    for s in range(2):
                        rbl = 16 * s
                        p0 = 16 * P + 8 * s   # first output row of sub-block
                        # vertical blur:
                        # vb[v] = (hb[v] + 3hb[v+1]) + (3hb[v+2] + hb[v+3])
                        vb = vbpool.tile([128, 17, HB_W], FP16)
                        nc.vector.tensor_tensor(
                            out=vu[:], in0=hb[:, rbl : rbl + 17, :],
                            in1=t3[:, rbl + 1 : rbl + 18, :], op=OP.add,
                        )
                        nc.vector.tensor_tensor(
                            out=vw[:], in0=t3[:, rbl + 2 : rbl + 19, :],
                            in1=hb[:, rbl + 3 : rbl + 20, :], op=OP.add,
                        )
                        nc.vector.tensor_tensor(
                            out=vb[:], in0=vu[:], in1=vw[:], op=OP.add,
                        )

                        for oc_h in range(2):
                            ot = opool.tile([128, 8, OW], FP16)
                            for ch in range(2):
                                ps = pspool.tile([128, 4, OW], FP32)
                                for t in range(9):
                                    kh, kw = t // 3, t % 3
                                    idx = t * 2 + oc_h
                                    nc.tensor.matmul(
                                        ps[:],
                                        wt[:, idx * 128 : (idx + 1) * 128],
                                        vb[:, 8 * ch + kh : 8 * ch + kh + 7 : 2,
                                           kw : kw + 255 : 2],
                                        start=(t == 0),
                                        stop=(t == 8),
                                    )
                                nc.scalar.activation(
                                    ot[:, 4 * ch : 4 * ch + 4, :], ps[:],
                                    AF.Prelu,
                                    bias=bt[:, oc_h : oc_h + 1],
                                    scale=SQRT2,
                                    alpha=al[:, 0:1],
                                )
                            nc.sync.dma_start(
                                y_d[img, 128 * oc_h : 128 * (oc_h + 1),
                                    p0 : p0 + 8, :],
                                ot[:],
                            )
    nc.finalize()
    return nc


_NC = None


def _get_nc():
    global _NC
    if _NC is None:
        _NC = _build_nc()
    return _NC


def kernel(x, weight, bias):
    x = np.asarray(x, dtype=np.float32)
    weight = np.asarray(weight, dtype=np.float32)
    bias = np.asarray(bias, dtype=np.float32)

    # host-side prep: fold wscale and the blur's 1/64 norm into the weights,
    # sqrt(2) gain and lr_mul into the bias; lay out lhsT tiles per (tap, half)
    w_eff = (weight * (WSCALE / 64.0)).astype(np.float16)  # [256,128,3,3]
    w_sb = np.empty((IC, 18 * 128), dtype=np.float16)
    for t in range(9):
        kh, kw = t // 3, t % 3
        for oc_h in range(2):
            idx = t * 2 + oc_h
            w_sb[:, idx * 128 : (idx + 1) * 128] = (
                w_eff[oc_h * 128 : (oc_h + 1) * 128, :, kh, kw].T
            )
    b_sb = (SQRT2 * bias).astype(np.float32).reshape(2, 128).T.copy()  # [128,2]

    x16 = x.astype(np.float16)
    nc = _get_nc()
    in_maps = [
        {
            "x": x16[c * B_PER_CORE : (c + 1) * B_PER_CORE],
            "w": w_sb,
            "b": b_sb,
        }
        for c in range(N_CORES)
    ]
    res = run_bass_kernel_spmd(nc, in_maps, core_ids=list(range(N_CORES)))
    y16 = np.concatenate([res.results[c]["y"] for c in range(N_CORES)], axis=0)
    return y16.astype(np.float32)



# revision 5
# speedup vs baseline: 1.5820x; 1.5820x over previous
"""Trainium2 Bass kernel for nn_ConvLayer_56453050139435.

Reference computation (StyleGAN2-style downsampling conv layer):
  1. depthwise 4x4 binomial blur ([1,3,3,1] outer [1,3,3,1] / 64) with pad 2
  2. 3x3 stride-2 conv, 128 -> 256 channels, weight scaled by 1/sqrt(fan_in)
  3. bias + leaky-relu(0.2) * sqrt(2), clamp +-256 (never binds: |out| < ~4)

Sharding: data-parallel over batch, 2 images per core across 8 cores.

Per-core pipeline (fp16 data path, fp32 PSUM accumulation):
  - one vertical blur tap [1,1] is folded into the conv weights, so the conv
    has 4x3=12 taps and the on-chip blur is (1+z_h)^3 (1+z_v)^2: five
    shifted-add stages a=x+x>>1, b=a+a>>1, hb=b+b>>1 (horizontal),
    u=hb+hb(down1), q=u+u(down1) (vertical), all plain tensor_tensor adds
    (2x DVE mode). The [1,3,3,1] weighting emerges from the add cascade.
  - stages are column-split between DVE (cols [0,QB)) and GpSimd (cols
    [QB,257)) so the two engines blur each strip in parallel; GpSimd
    recomputes a 2-3 col halo so the split is one-directional.
  - conv: 12-tap matmul accumulation in PSUM over the q tensor, rhs =
    stride-2 row/col access pattern; oc split in two 128-halves
  - epilogue: one ACT Prelu (scale=sqrt2, bias, alpha=0.2) PSUM->SBUF fp16
  - DMA out fp16, host-cast back to fp32
"""

import numpy as np

import concourse.bass as bass
import concourse.mybir as mybir
from concourse import bacc
from concourse.tile import TileContext
from concourse.bass_utils import run_bass_kernel_spmd

AF = mybir.ActivationFunctionType
OP = mybir.AluOpType
FP16 = mybir.dt.float16
FP32 = mybir.dt.float32

IC, OC, H, W = 128, 256, 256, 256
OH, OW = 128, 128
KS = 3
N_CORES = 8
B_PER_CORE = 2
SQRT2 = float(np.sqrt(2.0))
WSCALE = 1.0 / float(np.sqrt(KS * KS * IC))
LRELU_SLOPE = 0.2

NTAP = 12          # 4 vertical (w conv [1,1]) x 3 horizontal taps
XR = 36            # x rows per strip (32 + 4 halo)
QR = 34            # q rows per strip
NSTRIP = 8         # strips of 16 output rows per image
QW = 257           # valid q columns (0..256)
QB = 204           # q col split: DVE [0,QB), gpsimd [QB,257)
PW = QW - QB       # gpsimd q cols (53)


def _build_nc():
    nc = bacc.Bacc(None, target_bir_lowering=False)
    x_d = nc.dram_tensor("x", [B_PER_CORE, IC, H, W], FP16, kind="ExternalInput")
    w_d = nc.dram_tensor("w", [IC, 2 * NTAP * 128], FP16, kind="ExternalInput")
    b_d = nc.dram_tensor("b", [128, 2], FP32, kind="ExternalInput")
    y_d = nc.dram_tensor("y", [B_PER_CORE, OC, OH, OW], FP16, kind="ExternalOutput")

    with TileContext(nc) as tc:
        with (
            tc.tile_pool(name="const", bufs=1) as cpool,
            tc.tile_pool(name="xin", bufs=2) as xpool,
            tc.tile_pool(name="scr", bufs=1) as scrpool,
            tc.tile_pool(name="pscr", bufs=1) as ppool,
            tc.tile_pool(name="qq", bufs=2) as qpool,
            tc.tile_pool(name="out", bufs=4) as opool,
            tc.tile_pool(name="psum", bufs=8, space="PSUM") as pspool,
        ):
            wt = cpool.tile([128, 2 * NTAP * 128], FP16)
            bt = cpool.tile([128, 2], FP32)
            al = cpool.tile([128, 1], FP32)
            nc.sync.dma_start(wt[:], w_d[:])
            nc.sync.dma_start(bt[:], b_d[:])
            nc.vector.memset(al[:], LRELU_SLOPE)

            # DVE-only scratch (engine order serializes reuse across strips)
            s1 = scrpool.tile([128, XR, 260], FP16)   # a (cols 0:206) then hb
            s2 = scrpool.tile([128, XR, 260], FP16)   # b (cols 0:205) then u
            # gpsimd-only scratch, cols shifted by PBASE (buf col 0 == q col 202)
            pa = ppool.tile([128, XR, 58], FP16)      # a [202,259) then hb [202,257)
            pb = ppool.tile([128, XR, 56], FP16)      # b [202,258) then u [202,257)
            PBASE = QB - 2

            for img in range(B_PER_CORE):
                for S in range(NSTRIP):
                    p0 = 16 * S
                    lo = 2 * p0 - 2            # x row of xt row 0
                    xr0 = max(lo, 0)
                    xr1 = min(lo + XR, H)
                    ta, tb = xr0 - lo, xr1 - lo

                    # x strip: col i holds x col i-2; rows outside [ta,tb) and
                    # pad cols are zero.
                    xt = xpool.tile([128, XR, 260], FP16)
                    nc.gpsimd.memset(xt[:, ta:tb, 0:2], 0.0)
                    nc.gpsimd.memset(xt[:, ta:tb, 258:260], 0.0)
                    if ta > 0:
                        nc.gpsimd.memset(xt[:, 0:ta, :], 0.0)
                    if tb < XR:
                        nc.gpsimd.memset(xt[:, tb:XR, :], 0.0)
                    nc.sync.dma_start(
                        xt[:, ta:tb, 2:258], x_d[img, :, xr0:xr1, :]
                    )

                    q = qpool.tile([128, QR, 258], FP16)

                    # --- DVE blur arm: q cols [0, QB) ---
                    nc.vector.tensor_tensor(
                        out=s1[:, :, 0 : QB + 2], in0=xt[:, :, 0 : QB + 2],
                        in1=xt[:, :, 1 : QB + 3], op=OP.add,
                    )
                    nc.vector.tensor_tensor(
                        out=s2[:, :, 0 : QB + 1], in0=s1[:, :, 0 : QB + 1],
                        in1=s1[:, :, 1 : QB + 2], op=OP.add,
                    )
                    nc.vector.tensor_tensor(
                        out=s1[:, :, 0:QB], in0=s2[:, :, 0:QB],
                        in1=s2[:, :, 1 : QB + 1], op=OP.add,
                    )
                    nc.vector.tensor_tensor(
                        out=s2[:, 0 : XR - 1, 0:QB], in0=s1[:, 0 : XR - 1, 0:QB],
                        in1=s1[:, 1:XR, 0:QB], op=OP.add,
                    )
                    nc.vector.tensor_tensor(
                        out=q[:, :, 0:QB], in0=s2[:, 0:QR, 0:QB],
                        in1=s2[:, 1 : QR + 1, 0:QB], op=OP.add,
                    )

                    # --- gpsimd blur arm: q cols [QB, 257), 2-col halo ---
                    nc.gpsimd.tensor_tensor(
                        out=pa[:, :, 0:57], in0=xt[:, :, PBASE : PBASE + 57],
                        in1=xt[:, :, PBASE + 1 : PBASE + 58], op=OP.add,
                    )
                    nc.gpsimd.tensor_tensor(
                        out=pb[:, :, 0:56], in0=pa[:, :, 0:56],
                        in1=pa[:, :, 1:57], op=OP.add,
                    )
                    nc.gpsimd.tensor_tensor(
                        out=pa[:, :, 0:55], in0=pb[:, :, 0:55],
                        in1=pb[:, :, 1:56], op=OP.add,
                    )
                    nc.gpsimd.tensor_tensor(
                        out=pb[:, 0 : XR - 1, 0:55], in0=pa[:, 0 : XR - 1, 0:55],
                        in1=pa[:, 1:XR, 0:55], op=OP.add,
                    )
                    nc.gpsimd.tensor_tensor(
                        out=q[:, :, QB:QW], in0=pb[:, 0:QR, 2:55],
                        in1=pb[:, 1 : QR + 1, 2:55], op=OP.add,
                    )

                    # --- conv: 12 taps, 4 row-groups x 2 oc halves ---
                    for g in range(4):
                        for oc_h in range(2):
                            ps = pspool.tile([128, 4, OW], FP32)
                            for t in range(NTAP):
                                kh, kw = t // 3, t % 3
                                idx = t * 2 + oc_h
                                nc.tensor.matmul(
                                    ps[:],
                                    wt[:, idx * 128 : (idx + 1) * 128],
                                    q[:, 8 * g + kh : 8 * g + kh + 7 : 2,
                                      kw : kw + 255 : 2],
                                    start=(t == 0),
                                    stop=(t == NTAP - 1),
                                )
                            ot = opool.tile([128, 4, OW], FP16)
                            nc.scalar.activation(
                                ot[:], ps[:], AF.Prelu,
                                bias=bt[:, oc_h : oc_h + 1],
                                scale=SQRT2,
                                alpha=al[:, 0:1],
                            )
                            nc.sync.dma_start(
                                y_d[img, 128 * oc_h : 128 * (oc_h + 1),
                                    p0 + 4 * g : p0 + 4 * g + 4, :],
                                ot[:],
                            )
    nc.finalize()
    return nc


_NC = None


def _get_nc():
    global _NC
    if _NC is None:
        _NC = _build_nc()
    return _NC


def kernel(x, weight, bias):
    x = np.asarray(x, dtype=np.float32)
    weight = np.asarray(weight, dtype=np.float32)
    bias = np.asarray(bias, dtype=np.float32)

    # host prep: fold wscale and the blur's 1/64 norm into the weights, plus
    # one vertical blur tap [1,1]: w2[kh'] = w[kh'] + w[kh'-1], kh' in 0..3.
    w_eff = weight * (WSCALE / 64.0)                     # [256,128,3,3]
    w2 = np.zeros((OC, IC, 4, KS), dtype=np.float32)
    w2[:, :, 0:3, :] += w_eff
    w2[:, :, 1:4, :] += w_eff
    w2 = w2.astype(np.float16)
    w_sb = np.empty((IC, 2 * NTAP * 128), dtype=np.float16)
    for t in range(NTAP):
        kh, kw = t // 3, t % 3
        for oc_h in range(2):
            idx = t * 2 + oc_h
            w_sb[:, idx * 128 : (idx + 1) * 128] = (
                w2[oc_h * 128 : (oc_h + 1) * 128, :, kh, kw].T
            )
    b_sb = (SQRT2 * bias).astype(np.float32).reshape(2, 128).T.copy()  # [128,2]

    x16 = x.astype(np.float16)
    nc = _get_nc()
    in_maps = [
        {
            "x": x16[c * B_PER_CORE : (c + 1) * B_PER_CORE],
            "w": w_sb,
            "b": b_sb,
        }
        for c in range(N_CORES)
    ]
    res = run_bass_kernel_spmd(nc, in_maps, core_ids=list(range(N_CORES)))
    y16 = np.concatenate([res.results[c]["y"] for c in range(N_CORES)], axis=0)
    return y16.astype(np.float32)
